# revision 2
# baseline (speedup 1.0000x reference)
"""Trainium2 Bass kernel for the fused attention module.

8-core sharding: data-parallel over batch (B=2) x tensor-parallel over head
groups (32 heads -> 4 groups of 8). Core c handles batch c//4, head group c%4.
Each core computes QKV projection (its head slice), RoPE, full non-causal
attention for its 8 heads, and a partial output projection against its
W_out column slice; the host sums the 4 partials per batch.

Orientation notes (PE computes out = lhsT.T @ rhs, contraction on partitions):
- qT/kT produced as [f, n] (lhsT = W slice pre-transposed on host, rhs = xT)
- v produced as [n, f] (lhsT = xT tile, rhs = WvT)
- scoresT[j, i] per head (lhsT = kT j-tile, rhs = qT i-block); softmax over j
  (partitions) is handled by a ones-column on v producing the denominator row
- RoPE rotate_half is a partition shift, done as a small matmul with a
  constant block-diagonal permutation matrix R2T
- out[i, o] partial (lhsT = attT i-tile, rhs = W_outT slice)
"""

import os
import sys

sys.path.insert(0, "/opt/trn_rl_repo")

import numpy as np

import concourse.bass as bass  # noqa: F401
import concourse.mybir as mybir
import concourse.tile as tile
from concourse import bacc
from concourse.bass import ts
from concourse.bass_utils import run_bass_kernel_spmd

F32 = mybir.dt.float32
F32R = mybir.dt.float32r
# matmul operand dtype: float32r streams 1 col/cycle (4x faster than fp32 on
# the PE) at ~tf32 precision; set ATT_DT=f32 for full fp32.
DT = F32 if os.environ.get("ATT_DT") == "f32" else F32R

P = 128
NSEQ = 2048          # sequence length
CDIM = 2048          # model dim
HD = 64              # head dim
NHC = 8              # heads per core
KT = CDIM // P       # 16 contraction tiles
NB = 256             # n-block in the fused projection phase
NNB = NSEQ // NB     # 8
IB = 512             # i-block in attention
NIB = NSEQ // IB     # 4
JT = NSEQ // P       # 16 j-tiles
FQK = 2 * NHC * HD   # 1024 qk output features per core
MF = FQK // P        # 8 f-tiles (0-3 q, 4-7 k)
EXP_FUNC = mybir.ActivationFunctionType.Exp
SCALE = 1.0 / 8.0    # 1/sqrt(HD)

_CACHED_NC = None


def _build_nc():
    nc = bacc.Bacc(None)

    xt = nc.declare_dram_parameter("xt", [NNB, P, KT, NB], DT, isOutput=False)
    wqkt = nc.declare_dram_parameter("wqkt", [MF, P, KT, P], DT, isOutput=False)
    wvt = nc.declare_dram_parameter("wvt", [P, KT, 512], DT, isOutput=False)
    wot = nc.declare_dram_parameter("wot", [P, 4, CDIM], DT, isOutput=False)
    cos2t = nc.declare_dram_parameter("cos2t", [P, NSEQ], F32, isOutput=False)
    sin2t = nc.declare_dram_parameter("sin2t", [P, NSEQ], F32, isOutput=False)
    r2t = nc.declare_dram_parameter("r2t", [P, P], DT, isOutput=False)
    onesv = nc.declare_dram_parameter("onesv", [P, JT, NHC, 1], DT, isOutput=False)
    ones1 = nc.declare_dram_parameter("ones1", [1, HD], DT, isOutput=False)
    out_part = nc.declare_dram_parameter("out_part", [NSEQ, CDIM], F32, isOutput=True)

    # DRAM spill for rotated qT/kT: [mf, p, n]
    qkspill = nc.dram_tensor("qkspill", [MF, P, NSEQ], DT)

    # ATT_REPEAT>1 duplicates the body inside one NEFF (timing experiments:
    # device-time = delta between repeat counts; results are idempotent)
    n_repeat = int(os.environ.get("ATT_REPEAT", "1"))

    with tile.TileContext(nc) as tc, nc.allow_low_precision("fp32r matmul kernel"):
        for _rep in range(n_repeat):
            _kernel_body(nc, tc, xt, wqkt, wvt, wot, cos2t, sin2t, r2t,
                         onesv, ones1, out_part, qkspill)

    nc.compile()
    return nc


def _kernel_body(nc, tc, xt, wqkt, wvt, wot, cos2t, sin2t, r2t,
                 onesv, ones1, out_part, qkspill):
        with tc.tile_pool(name="persist", bufs=1) as persist:
            ones1_sb = persist.tile([1, HD], DT)
            nc.sync.dma_start(out=ones1_sb, in_=ones1[:, :])

            # ---- Phase AB: fused v + q/k projection + RoPE. One pass over x
            # (streamed in NB-column blocks); q/k (rotated) spill to DRAM,
            # v stays resident. ----
            v1p_cm = tc.tile_pool(name="v1p", bufs=1)
            v1p = v1p_cm.__enter__()
            xtp_cm = tc.tile_pool(name="xtp", bufs=2)
            xtp = xtp_cm.__enter__()
            bconst_cm = tc.tile_pool(name="bconst", bufs=1)
            bconst = bconst_cm.__enter__()
            aconst_cm = tc.tile_pool(name="aconst", bufs=1)
            aconst = aconst_cm.__enter__()

            # Interleave wvt and xt[0] loads per k-tile so the first
            # accumulation chain starts as soon as the first k-slices land.
            wvt_sb = aconst.tile([P, KT, 512], DT)
            xt_first = xtp.tile([P, KT, NB], DT, tag="xt")
            xt0_r = xt[0]
            for kc in range(KT):
                nc.sync.dma_start(out=wvt_sb[:, kc, :], in_=wvt[:, kc, :])
                nc.sync.dma_start(out=xt_first[:, kc, :], in_=xt0_r[:, kc, :])
            v1_sb = v1p.tile([P, JT, NHC, HD + 1], DT)
            r2_sb = bconst.tile([P, P], DT)
            wqk_sb = bconst.tile([P, KT, MF, P], DT)
            nc.sync.dma_start(
                out=v1_sb[:, :, :, HD:HD + 1],
                in_=onesv[:, :, :, :],
            )
            nc.sync.dma_start(out=r2_sb, in_=r2t[:, :])
            for mf in range(MF):
                nc.sync.dma_start(
                    out=wqk_sb[:, :, mf, :],
                    in_=wqkt[mf],
                )

            with tc.tile_pool(name="csp", bufs=2) as csp, \
                 tc.tile_pool(name="stg", bufs=3) as stg, \
                 tc.tile_pool(name="psv", bufs=2, space="PSUM") as psv, \
                 tc.tile_pool(name="psqk", bufs=3, space="PSUM") as psqk, \
                 tc.tile_pool(name="psrot", bufs=2, space="PSUM") as psrot:
                for nb in range(NNB):
                    if nb == 0:
                        xt_t = xt_first
                    else:
                        xt_t = xtp.tile([P, KT, NB], DT, tag="xt")
                        nc.sync.dma_start(out=xt_t, in_=xt[nb])
                    nsl = ts(nb, NB)
                    cos_sb = csp.tile([P, NB], F32, tag="cos")
                    sin_sb = csp.tile([P, NB], F32, tag="sin")
                    nc.sync.dma_start(out=cos_sb, in_=cos2t[:, nsl])
                    nc.sync.dma_start(out=sin_sb, in_=sin2t[:, nsl])
                    # v projection for this n-block
                    for t4 in range(NB // P):
                        nt = nb * (NB // P) + t4
                        vp = psv.tile([P, 512], F32, tag="vp")
                        for kc in range(KT):
                            nc.tensor.matmul(
                                vp,
                                xt_t[:, kc, ts(t4, P)],
                                wvt_sb[:, kc, :],
                                start=(kc == 0),
                                stop=(kc == KT - 1),
                            )
                        nc.scalar.copy(
                            out=v1_sb[:, nt, :, 0:HD],
                            in_=vp.rearrange("p (h d) -> p h d", h=NHC),
                        )
                    # q/k projection + rope for this n-block
                    for mf in range(MF):
                        qp = psqk.tile([P, NB], F32, tag="qp")
                        for kc in range(KT):
                            nc.tensor.matmul(
                                qp,
                                wqk_sb[:, kc, mf, :],
                                xt_t[:, kc, :],
                                start=(kc == 0),
                                stop=(kc == KT - 1),
                            )
                        qa = stg.tile([P, NB], DT, tag="qa")
                        nc.scalar.copy(out=qa, in_=qp)
                        rp = psrot.tile([P, NB], F32, tag="rp")
                        nc.tensor.matmul(rp, r2_sb, qa, start=True, stop=True)
                        t1 = stg.tile([P, NB], F32, tag="t1")
                        nc.vector.tensor_mul(out=t1, in0=qa.bitcast(F32), in1=cos_sb)
                        t2 = stg.tile([P, NB], F32, tag="t2")
                        nc.vector.tensor_mul(out=t2, in0=rp, in1=sin_sb)
                        qk_out = stg.tile([P, NB], DT, tag="qko")
                        nc.vector.tensor_add(out=qk_out, in0=t1, in1=t2)
                        nc.gpsimd.dma_start(out=qkspill[mf, :, nsl], in_=qk_out)

            aconst_cm.__exit__(None, None, None)
            bconst_cm.__exit__(None, None, None)
            xtp_cm.__exit__(None, None, None)

            if os.environ.get("ATT_PHASE") == "ab":
                # timing-only build: stop after projections; emit a token
                # output write so the NEFF has its declared output
                dummy = v1p.tile([P, NHC, HD], F32, tag="dummy")
                nc.vector.tensor_copy(out=dummy, in_=v1_sb[:, 0, :, 0:HD].bitcast(F32))
                nc.sync.dma_start(out=out_part[0:P, 0:NHC * HD], in_=dummy.rearrange("p h d -> p (h d)"))
                v1p_cm.__exit__(None, None, None)
                return

            # ---- Phase C+D: attention per head, with the output projection
            # interleaved per i-half so its matmuls fill the PE while the
            # scalar engine (exp) is the bottleneck ----
            attp_cm = tc.tile_pool(name="attp", bufs=1)
            attp = attp_cm.__enter__()
            att_sb = attp.tile([P, 4, NSEQ], DT)
            dconst_cm = tc.tile_pool(name="dconst", bufs=1)
            dconst = dconst_cm.__enter__()
            with tc.tile_pool(name="qkp", bufs=2) as qkp, \
                 tc.tile_pool(name="expp", bufs=6) as expp, \
                 tc.tile_pool(name="smal", bufs=4) as smal, \
                 tc.tile_pool(name="evp", bufs=3) as evp, \
                 tc.tile_pool(name="pssc0", bufs=1, space="PSUM") as pssc0, \
                 tc.tile_pool(name="pssc1", bufs=1, space="PSUM") as pssc1, \
                 tc.tile_pool(name="psav", bufs=1, space="PSUM") as psav, \
                 tc.tile_pool(name="psbc", bufs=1, space="PSUM") as psbc, \
                 tc.tile_pool(name="psd", bufs=2, space="PSUM") as psd:
                # first pair's q/k ahead of the (larger) wot load
                qt0 = qkp.tile([P, NSEQ], DT, tag="qt")
                nc.sync.dma_start(out=qt0, in_=qkspill[0])
                kt0 = qkp.tile([P, NSEQ], DT, tag="kt")
                nc.sync.dma_start(out=kt0, in_=qkspill[4])
                wot_sb = dconst.tile([P, 4, CDIM], DT)
                for ct in range(4):
                    nc.sync.dma_start(out=wot_sb[:, ct, :], in_=wot[:, ct, :])
                for ib in range(NIB):
                    isl = ts(ib, IB)
                    for t in range(4):
                        if ib == 0 and t == 0:
                            qt_sb, kt_sb = qt0, kt0
                        else:
                            qt_sb = qkp.tile([P, NSEQ], DT, tag="qt")
                            nc.sync.dma_start(out=qt_sb, in_=qkspill[t])
                            kt_sb = qkp.tile([P, NSEQ], DT, tag="kt")
                            nc.sync.dma_start(out=kt_sb, in_=qkspill[4 + t])
                        # scores for BOTH heads of the pair, interleaved so the
                        # two K=64 matmuls run concurrently in the PE array's
                        # two row halves (lhsT base partitions 0 and 64)
                        expq = ([], [])
                        for grp in range(8):
                            qi, qs = divmod(grp, 2)
                            if qs == 0:
                                for h2 in range(2):
                                    exp_q = expp.tile([P, 4, IB], DT, tag="exp")
                                    expq[h2].append(exp_q)
                            sc0 = pssc0.tile([P, 2, IB], F32, tag="sc0")
                            sc1 = pssc1.tile([P, 2, IB], F32, tag="sc1")
                            for j2 in range(2):
                                jt = grp * 2 + j2
                                nc.tensor.matmul(
                                    sc0[:, j2, :],
                                    kt_sb[0:HD, ts(jt, P)],
                                    qt_sb[0:HD, isl],
                                    start=True,
                                    stop=True,
                                )
                                nc.tensor.matmul(
                                    sc1[:, j2, :],
                                    kt_sb[HD:P, ts(jt, P)],
                                    qt_sb[HD:P, isl],
                                    start=True,
                                    stop=True,
                                )
                            nc.scalar.activation(
                                out=expq[0][qi][:, ts(qs, 2), :],
                                in_=sc0,
                                func=EXP_FUNC,
                                scale=SCALE,
                            )
                            nc.scalar.activation(
                                out=expq[1][qi][:, ts(qs, 2), :],
                                in_=sc1,
                                func=EXP_FUNC,
                                scale=SCALE,
                            )
                        # attn @ v (ones-column denominator row) + normalize
                        for h2 in range(2):
                            hb = HD * h2
                            h = 2 * t + h2
                            av = psav.tile([HD + 1, 512], F32, tag="av")
                            for jt in range(JT):
                                nc.tensor.matmul(
                                    av,
                                    v1_sb[:, jt, h, :],
                                    expq[h2][jt // 4][:, jt % 4, :],
                                    start=(jt == 0),
                                    stop=(jt == JT - 1),
                                )
                            rd = smal.tile([1, IB], DT, tag="rd")
                            nc.vector.reciprocal(out=rd, in_=av[HD:HD + 1, :])
                            bc = psbc.tile([HD, IB], F32, tag="bc")
                            nc.tensor.matmul(bc, ones1_sb, rd, start=True, stop=True)
                            bc_sb = smal.tile([HD, IB], F32, tag="bcs")
                            nc.vector.tensor_copy(out=bc_sb, in_=bc)
                            nc.vector.tensor_mul(
                                out=att_sb[hb:hb + HD, t, isl],
                                in0=av[0:HD, :],
                                in1=bc_sb,
                            )
                    # output projection for this i-block (att columns complete)
                    if os.environ.get("ATT_PHASE") == "abc":
                        continue
                    for it in range(4 * ib, 4 * ib + 4):
                        for ob in range(4):
                            op = psd.tile([P, 512], F32, tag="op")
                            for ct in range(4):
                                nc.tensor.matmul(
                                    op,
                                    att_sb[:, ct, ts(it, P)],
                                    wot_sb[:, ct, ts(ob, 512)],
                                    start=(ct == 0),
                                    stop=(ct == 3),
                                )
                            o_sb = evp.tile([P, 512], F32, tag="osb")
                            nc.vector.tensor_copy(out=o_sb, in_=op)
                            nc.sync.dma_start(
                                out=out_part[ts(it, P), ts(ob, 512)], in_=o_sb
                            )
                if os.environ.get("ATT_PHASE") == "abc":
                    o_dummy = evp.tile([P, 512], F32, tag="osb")
                    nc.vector.tensor_copy(out=o_dummy, in_=att_sb[:, 0, 0:512].bitcast(F32))
                    nc.sync.dma_start(out=out_part[0:P, 0:512], in_=o_dummy)
            dconst_cm.__exit__(None, None, None)
            attp_cm.__exit__(None, None, None)
            v1p_cm.__exit__(None, None, None)


def _rot_matrix():
    r = np.zeros((HD, HD), dtype=np.float32)
    for d in range(32):
        r[d, d + 32] = -1.0
    for d in range(32, HD):
        r[d, d - 32] = 1.0
    r2 = np.zeros((P, P), dtype=np.float32)
    r2[0:HD, 0:HD] = r
    r2[HD:P, HD:P] = r
    return np.ascontiguousarray(r2.T)


def _core_inputs(x, cos_t, sin_t, W_qkv, W_out, core):
    b, g = divmod(core, 4)
    hs = g * NHC * HD  # feature offset of this head group (512 per group)

    xT = np.ascontiguousarray(x[b].T)  # [c, n]
    xt = np.ascontiguousarray(
        xT.reshape(KT, P, NNB, NB).transpose(2, 1, 0, 3)
    )

    Wq = W_qkv[hs:hs + 512]
    Wk = W_qkv[CDIM + hs:CDIM + hs + 512]
    Wv = W_qkv[2 * CDIM + hs:2 * CDIM + hs + 512]
    WqkT = np.ascontiguousarray(np.concatenate([Wq, Wk], axis=0).T)  # [c, 1024]
    wqkt = np.ascontiguousarray(
        WqkT.reshape(KT, P, MF, P).transpose(2, 1, 0, 3)
    )
    WvT = np.ascontiguousarray(Wv.T)  # [c, 512]
    wvt = np.ascontiguousarray(WvT.reshape(KT, P, 512).transpose(1, 0, 2))
    WoT = np.ascontiguousarray(W_out[:, hs:hs + 512].T)  # [c-slice 512, o 2048]
    wot = np.ascontiguousarray(WoT.reshape(4, P, CDIM).transpose(1, 0, 2))

    return {
        "xt": xt,
        "wqkt": wqkt,
        "wvt": wvt,
        "wot": wot,
        "cos2t": cos_t,
        "sin2t": sin_t,
        "r2t": _ROT,
        "onesv": _ONESV,
        "ones1": _ONES1,
    }


_ROT = _rot_matrix()
_ONESV = np.ones((P, JT, NHC, 1), dtype=np.float32)
_ONES1 = np.ones((1, HD), dtype=np.float32)


def kernel(x, freqs, W_qkv, W_out):
    global _CACHED_NC
    x = np.asarray(x, dtype=np.float32)
    freqs = np.asarray(freqs, dtype=np.float32)
    W_qkv = np.asarray(W_qkv, dtype=np.float32)
    W_out = np.asarray(W_out, dtype=np.float32)

    if _CACHED_NC is None:
        _CACHED_NC = _build_nc()
    nc = _CACHED_NC

    cos_t = np.ascontiguousarray(np.tile(np.cos(freqs.T), (2, 1)))  # [128, n]
    sin_t = np.ascontiguousarray(np.tile(np.sin(freqs.T), (2, 1)))

    in_maps = [
        _core_inputs(x, cos_t, sin_t, W_qkv, W_out, core) for core in range(8)
    ]
    trace = os.environ.get("ATT_TRACE") == "1"
    tdir = os.environ.get("ATT_TRACE_DIR") or None
    res = run_bass_kernel_spmd(
        nc, in_maps, core_ids=list(range(8)), trace=trace, tmpdir=tdir
    )
    if trace and res.exec_time_ns is not None:
        print(f"HW exec time: {res.exec_time_ns} ns")

    out = np.empty((2, NSEQ, CDIM), dtype=np.float32)
    for b in range(2):
        acc = np.zeros((NSEQ, CDIM), dtype=np.float64)
        for g in range(4):
            acc += res.results[4 * b + g]["out_part"]
        out[b] = acc.astype(np.float32)
    return out



# revision 7
# speedup vs baseline: 1.4744x; 1.4744x over previous
"""Trainium2 Bass kernel for the fused attention module (bf16 matmul path).

8-core sharding: data-parallel over batch (B=2) x tensor-parallel over head
groups (32 heads -> 4 groups of 8). Core c handles batch c//4, head group c%4.
Each core computes QKV projection (its head slice), RoPE, full non-causal
attention for its 8 heads, and a partial output projection against its
W_out column slice; the host sums the 4 partials per batch.

All matmul operands are bf16 (1 col/cycle on the PE vs 2 cycles/col for the
fp32 path measured on HW; fp32r degrades to fp32-HIGH on this silicon).
PSUM accumulation stays fp32. q/k stay resident in SBUF (no DRAM spill).

Phase AB: v projection (n on partitions, for av lhsT), then q/k projection
(features on partitions) + RoPE, writing qk_sb in place.
Phase C: per (i-block, head-pair): scores as two row-tiled K=64 matmuls
(tile_position (0,0)/(64,0) via base partitions -> concurrent halves), exp on
the scalar engine (the only engine with the activation LUT -> it is the
critical engine; everything else is kept off it), attn@v with a ones-column
producing the softmax denominator row, normalize via ones-outer-product
broadcast + reciprocal_approx_fast, and the output projection interleaved
into the next i-block's score phase so the PE fills while the scalar engine
streams exp.
"""

import os
import sys

sys.path.insert(0, "/opt/trn_rl_repo")

import numpy as np
import ml_dtypes

import concourse.bass as bass  # noqa: F401
import concourse.mybir as mybir
import concourse.tile as tile
from concourse import bacc
from concourse.bass import ts
from concourse.bass_utils import run_bass_kernel_spmd

F32 = mybir.dt.float32
BF = mybir.dt.bfloat16
NPBF = ml_dtypes.bfloat16

P = 128
NSEQ = 2048          # sequence length
CDIM = 2048          # model dim
HD = 64              # head dim
NHC = 8              # heads per core
KT = CDIM // P       # 16 contraction tiles
IB = 512             # i-block in attention
NIB = NSEQ // IB     # 4
JT = NSEQ // P       # 16 j-tiles
MF = 8               # qk f-tiles (0-3 q, 4-7 k)
NXC = 8              # x chunk tiles (2 kc each)
EXP_FUNC = mybir.ActivationFunctionType.Exp
SCALE = 1.0 / 8.0    # 1/sqrt(HD)

_CACHED_NC = None


def _build_nc():
    nc = bacc.Bacc(None)

    xt = nc.declare_dram_parameter("xt", [KT, P, NSEQ], BF, isOutput=False)
    wqkt = nc.declare_dram_parameter("wqkt", [P, KT, MF, P], BF, isOutput=False)
    wvt = nc.declare_dram_parameter("wvt", [P, KT, 512], BF, isOutput=False)
    wot = nc.declare_dram_parameter("wot", [P, 4, CDIM], BF, isOutput=False)
    cos2t = nc.declare_dram_parameter("cos2t", [P, NSEQ], F32, isOutput=False)
    sin2t = nc.declare_dram_parameter("sin2t", [P, NSEQ], F32, isOutput=False)
    r2t = nc.declare_dram_parameter("r2t", [P, P], BF, isOutput=False)
    onesv = nc.declare_dram_parameter("onesv", [P, JT, NHC, 1], BF, isOutput=False)
    onesb = nc.declare_dram_parameter("onesb", [P, HD], BF, isOutput=False)
    out_part = nc.declare_dram_parameter("out_part", [NSEQ, CDIM], F32, isOutput=True)

    n_repeat = int(os.environ.get("ATT_REPEAT", "1"))

    with tile.TileContext(nc) as tc, nc.allow_low_precision("bf16 matmul kernel"):
        for _rep in range(n_repeat):
            _kernel_body(nc, tc, xt, wqkt, wvt, wot, cos2t, sin2t, r2t,
                         onesv, onesb, out_part)

    nc.compile()
    return nc


def _kernel_body(nc, tc, xt, wqkt, wvt, wot, cos2t, sin2t, r2t,
                 onesv, onesb, out_part):
    with tc.tile_pool(name="persist", bufs=1) as persist:
        qk_sb = persist.tile([P, MF, NSEQ], BF)
        v1_sb = persist.tile([P, JT, NHC, HD + 1], BF)
        ones_sb = persist.tile([P, HD], BF)
        nc.sync.dma_start(out=ones_sb, in_=onesb[:, :])
        nc.sync.dma_start(out=v1_sb[:, :, :, HD:HD + 1], in_=onesv[:, :, :, :])

        # ---------------- Phase AB: projections + RoPE ----------------
        xp_cm = tc.tile_pool(name="xp", bufs=NXC)
        xp = xp_cm.__enter__()
        wp_cm = tc.tile_pool(name="wp", bufs=1)
        wp = wp_cm.__enter__()
        csp_cm = tc.tile_pool(name="csp", bufs=1)
        csp = csp_cm.__enter__()

        wvt_sb = wp.tile([P, KT, 512], BF)
        nc.sync.dma_start(out=wvt_sb, in_=wvt[:, :, :])
        xch = []
        for c in range(NXC):
            xc = xp.tile([P, 2, NSEQ], BF, tag="xc")
            for k in range(2):
                nc.sync.dma_start(out=xc[:, k, :], in_=xt[2 * c + k])
            xch.append(xc)
        wqk_sb = wp.tile([P, KT, MF, P], BF)
        nc.sync.dma_start(out=wqk_sb, in_=wqkt[:, :, :, :])
        cos_sb = csp.tile([P, NSEQ], F32)
        sin_sb = csp.tile([P, NSEQ], F32)
        r2_sb = csp.tile([P, P], BF)
        nc.sync.dma_start(out=cos_sb, in_=cos2t[:, :])
        nc.sync.dma_start(out=sin_sb, in_=sin2t[:, :])
        nc.sync.dma_start(out=r2_sb, in_=r2t[:, :])

        def xsl(kc, nslice):
            return xch[kc // 2][:, kc % 2, nslice]

        # v projection: [n on partitions, v-features free]
        with tc.tile_pool(name="psv", bufs=2, space="PSUM") as psv:
            for nt in range(JT):
                vp = psv.tile([P, 512], F32, tag="vp")
                for kc in range(KT):
                    nc.tensor.matmul(
                        vp,
                        xsl(kc, ts(nt, P)),
                        wvt_sb[:, kc, :],
                        start=(kc == 0),
                        stop=(kc == KT - 1),
                    )
                nc.scalar.copy(
                    out=v1_sb[:, nt, :, 0:HD],
                    in_=vp.rearrange("p (h d) -> p h d", h=NHC),
                )

        # q/k projection + RoPE: [qk-features on partitions, n free]
        with tc.tile_pool(name="psqk", bufs=2, space="PSUM") as psqk, \
             tc.tile_pool(name="psrot", bufs=2, space="PSUM") as psrot, \
             tc.tile_pool(name="stg", bufs=2) as stg, \
             tc.tile_pool(name="stg2", bufs=2) as stg2:
            for mf in range(MF):
                for nh in range(2):
                    nsl = ts(nh, 1024)
                    qp = psqk.tile([P, 2, 512], F32, tag="qp")
                    for kc in range(KT):
                        for q2 in range(2):
                            nc.tensor.matmul(
                                qp[:, q2, :],
                                wqk_sb[:, kc, mf, :],
                                xsl(kc, ts(2 * nh + q2, 512)),
                                start=(kc == 0),
                                stop=(kc == KT - 1),
                            )
                    qpf = qp.rearrange("p a b -> p (a b)")
                    qa = stg.tile([P, 1024], BF, tag="qa")
                    nc.scalar.copy(out=qa, in_=qpf)
                    rp = psrot.tile([P, 2, 512], F32, tag="rp")
                    for q2 in range(2):
                        nc.tensor.matmul(
                            rp[:, q2, :], r2_sb, qa[:, ts(q2, 512)],
                            start=True, stop=True,
                        )
                    t1 = stg2.tile([P, 1024], F32, tag="t1")
                    nc.vector.tensor_mul(out=t1, in0=qpf, in1=cos_sb[:, nsl])
                    t2 = stg2.tile([P, 1024], F32, tag="t2")
                    nc.vector.tensor_mul(
                        out=t2, in0=rp.rearrange("p a b -> p (a b)"),
                        in1=sin_sb[:, nsl],
                    )
                    nc.vector.tensor_add(out=qk_sb[:, mf, nsl], in0=t1, in1=t2)

        csp_cm.__exit__(None, None, None)
        wp_cm.__exit__(None, None, None)
        xp_cm.__exit__(None, None, None)

        # ---------------- Phase C: attention + output projection ----------------
        wotp_cm = tc.tile_pool(name="wotp", bufs=1)
        wotp = wotp_cm.__enter__()
        attp_cm = tc.tile_pool(name="attp", bufs=1)
        attp = attp_cm.__enter__()
        wot_sb = wotp.tile([P, 4, CDIM], BF)
        for ct in range(4):
            nc.sync.dma_start(out=wot_sb[:, ct, :], in_=wot[:, ct, :])
        att_sb = attp.tile([P, 4, NSEQ], BF)

        with tc.tile_pool(name="expp", bufs=10) as expp, \
             tc.tile_pool(name="smal", bufs=6) as smal, \
             tc.tile_pool(name="osb", bufs=3) as osb, \
             tc.tile_pool(name="pssc0", bufs=1, space="PSUM") as pssc0, \
             tc.tile_pool(name="pssc1", bufs=1, space="PSUM") as pssc1, \
             tc.tile_pool(name="psav", bufs=2, space="PSUM") as psav, \
             tc.tile_pool(name="psd", bufs=2, space="PSUM") as psd:

            def emit_outproj_quarter(ib, q):
                it = 4 * ib + q
                for ob in range(4):
                    op = psd.tile([P, 512], F32, tag="op")
                    for ct in range(4):
                        nc.tensor.matmul(
                            op,
                            att_sb[:, ct, ts(it, P)],
                            wot_sb[:, ct, ts(ob, 512)],
                            start=(ct == 0),
                            stop=(ct == 3),
                        )
                    o_sb = osb.tile([P, 512], F32, tag="osb")
                    nc.vector.tensor_copy(out=o_sb, in_=op)
                    nc.sync.dma_start(
                        out=out_part[ts(it, P), ts(ob, 512)], in_=o_sb
                    )

            for ib in range(NIB):
                isl = ts(ib, IB)
                for t in range(4):
                    exps = ([None] * 8, [None] * 8)
                    avt0 = psav.tile([HD + 1, IB], F32, tag="av")
                    avt1 = psav.tile([HD + 1, IB], F32, tag="av")
                    avt = (avt0, avt1)
                    for grp in range(8):
                        sc0 = pssc0.tile([P, 2, IB], F32, tag="sc0")
                        sc1 = pssc1.tile([P, 2, IB], F32, tag="sc1")
                        for j2 in range(2):
                            jt = 2 * grp + j2
                            nc.tensor.matmul(
                                sc0[:, j2, :],
                                qk_sb[0:HD, 4 + t, ts(jt, P)],
                                qk_sb[0:HD, t, isl],
                                start=True,
                                stop=True,
                            )
                            nc.tensor.matmul(
                                sc1[:, j2, :],
                                qk_sb[HD:P, 4 + t, ts(jt, P)],
                                qk_sb[HD:P, t, isl],
                                start=True,
                                stop=True,
                            )
                        e0 = expp.tile([P, 2, IB], BF, tag="exp")
                        nc.scalar.activation(out=e0, in_=sc0, func=EXP_FUNC,
                                             scale=SCALE)
                        e1 = expp.tile([P, 2, IB], BF, tag="exp")
                        nc.scalar.activation(out=e1, in_=sc1, func=EXP_FUNC,
                                             scale=SCALE)
                        exps[0][grp] = e0
                        exps[1][grp] = e1
                        # PE fills with av of the previous group while the
                        # scalar engine streams exp of this group
                        if grp > 0:
                            for h2 in range(2):
                                h = 2 * t + h2
                                for j2 in range(2):
                                    jt = 2 * (grp - 1) + j2
                                    nc.tensor.matmul(
                                        avt[h2],
                                        v1_sb[:, jt, h, :],
                                        exps[h2][grp - 1][:, j2, :],
                                        start=(jt == 0),
                                        stop=False,
                                        skip_group_check=True,
                                    )
                        # output projection of the previous i-block, spread
                        # one i-tile per head-pair slot
                        if grp == 2 and ib > 0:
                            emit_outproj_quarter(ib - 1, t)
                    for h2 in range(2):
                        h = 2 * t + h2
                        for j2 in range(2):
                            jt = 14 + j2
                            nc.tensor.matmul(
                                avt[h2],
                                v1_sb[:, jt, h, :],
                                exps[h2][7][:, j2, :],
                                start=False,
                                stop=(jt == JT - 1),
                                skip_group_check=True,
                            )
                    # softmax normalize: denominator row 64 of av
                    for h2 in range(2):
                        hb = HD * h2
                        denb = smal.tile([HD + 1, IB], BF, tag="denb")
                        nc.vector.tensor_copy(
                            out=denb[HD:HD + 1, :],
                            in_=avt[h2][HD:HD + 1, :],
                        )
                        bc = psd.tile([P, IB], F32, tag="op")
                        nc.tensor.matmul(
                            bc[0:HD, :],
                            ones_sb[HD:HD + 1, :],
                            denb[HD:HD + 1, :],
                            start=True,
                            stop=True,
                        )
                        bcr = smal.tile([HD, IB], F32, tag="bcr")
                        nc.vector.reciprocal_approx_fast(
                            out=bcr, in_=bc[0:HD, :]
                        )
                        nc.vector.tensor_mul(
                            out=att_sb[hb:hb + HD, t, isl],
                            in0=avt[h2][0:HD, :],
                            in1=bcr,
                        )
            for q in range(4):
                emit_outproj_quarter(NIB - 1, q)

        attp_cm.__exit__(None, None, None)
        wotp_cm.__exit__(None, None, None)


def _rot_matrix():
    r = np.zeros((HD, HD), dtype=np.float32)
    for d in range(32):
        r[d, d + 32] = -1.0
    for d in range(32, HD):
        r[d, d - 32] = 1.0
    r2 = np.zeros((P, P), dtype=np.float32)
    r2[0:HD, 0:HD] = r
    r2[HD:P, HD:P] = r
    return np.ascontiguousarray(r2.T).astype(NPBF)


def _core_inputs(x, cos_t, sin_t, W_qkv, W_out, core):
    b, g = divmod(core, 4)
    hs = g * NHC * HD  # feature offset of this head group (512 per group)

    xT = np.ascontiguousarray(x[b].T).astype(NPBF)  # [c, n]
    xt = np.ascontiguousarray(xT.reshape(KT, P, NSEQ))

    Wq = W_qkv[hs:hs + 512]
    Wk = W_qkv[CDIM + hs:CDIM + hs + 512]
    Wv = W_qkv[2 * CDIM + hs:2 * CDIM + hs + 512]
    WqkT = np.concatenate([Wq, Wk], axis=0).T  # [c, 1024]
    wqkt = np.ascontiguousarray(
        WqkT.reshape(KT, P, MF, P).transpose(1, 0, 2, 3)
    ).astype(NPBF)
    WvT = Wv.T  # [c, 512]
    wvt = np.ascontiguousarray(
        WvT.reshape(KT, P, 512).transpose(1, 0, 2)
    ).astype(NPBF)
    WoT = W_out[:, hs:hs + 512].T  # [c-slice 512, o 2048]
    wot = np.ascontiguousarray(
        WoT.reshape(4, P, CDIM).transpose(1, 0, 2)
    ).astype(NPBF)

    return {
        "xt": xt,
        "wqkt": wqkt,
        "wvt": wvt,
        "wot": wot,
        "cos2t": cos_t,
        "sin2t": sin_t,
        "r2t": _ROT,
        "onesv": _ONESV,
        "onesb": _ONESB,
    }


_ROT = _rot_matrix()
_ONESV = np.ones((P, JT, NHC, 1), dtype=NPBF)
_ONESB = np.ones((P, HD), dtype=NPBF)


def kernel(x, freqs, W_qkv, W_out):
    global _CACHED_NC
    x = np.asarray(x, dtype=np.float32)
    freqs = np.asarray(freqs, dtype=np.float32)
    W_qkv = np.asarray(W_qkv, dtype=np.float32)
    W_out = np.asarray(W_out, dtype=np.float32)

    if _CACHED_NC is None:
        _CACHED_NC = _build_nc()
    nc = _CACHED_NC

    cos_t = np.ascontiguousarray(np.tile(np.cos(freqs.T), (2, 1)))  # [128, n]
    sin_t = np.ascontiguousarray(np.tile(np.sin(freqs.T), (2, 1)))

    in_maps = [
        _core_inputs(x, cos_t, sin_t, W_qkv, W_out, core) for core in range(8)
    ]
    trace = os.environ.get("ATT_TRACE") == "1"
    tdir = os.environ.get("ATT_TRACE_DIR") or None
    res = run_bass_kernel_spmd(
        nc, in_maps, core_ids=list(range(8)), trace=trace, tmpdir=tdir
    )
    if trace and res.exec_time_ns is not None:
        print(f"HW exec time: {res.exec_time_ns} ns")

    out = np.empty((2, NSEQ, CDIM), dtype=np.float32)
    for b in range(2):
        acc = np.zeros((NSEQ, CDIM), dtype=np.float64)
        for g in range(4):
            acc += res.results[4 * b + g]["out_part"]
        out[b] = acc.astype(np.float32)
    return out


# revision 10
# speedup vs baseline: 1.6448x; 1.1156x over previous
"""Trainium2 Bass kernel for the fused attention module (bf16 matmul path).

8-core sharding: data-parallel over batch (B=2) x tensor-parallel over head
groups (32 heads -> 4 groups of 8). Core c handles batch c//4, head group c%4.
Each core computes QKV projection (its head slice), RoPE, full non-causal
attention for its 8 heads, and a partial output projection against its
W_out column slice; the host sums the 4 partials per batch.

All matmul operands are bf16 (1 col/cycle on the PE vs 2 cycles/col for the
fp32 path measured on HW; fp32r degrades to fp32-HIGH on this silicon).
PSUM accumulation stays fp32. q/k stay resident in SBUF (no DRAM spill).

Phase AB: v projection (n on partitions, for av lhsT), then q/k projection
(features on partitions) + RoPE, writing qk_sb in place.
Phase C: per (i-block, head-pair): scores as two row-tiled K=64 matmuls
(tile_position (0,0)/(64,0) via base partitions -> concurrent halves), exp on
the scalar engine (the only engine with the activation LUT -> it is the
critical engine; everything else is kept off it), attn@v with a ones-column
producing the softmax denominator row, normalize via ones-outer-product
broadcast + reciprocal_approx_fast, and the output projection interleaved
into the next i-block's score phase so the PE fills while the scalar engine
streams exp.
"""

import os
import sys

sys.path.insert(0, "/opt/trn_rl_repo")

import numpy as np
import ml_dtypes

import concourse.bass as bass  # noqa: F401
import concourse.mybir as mybir
import concourse.tile as tile
from concourse import bacc
from concourse.bass import ts
from concourse.bass_utils import run_bass_kernel_spmd

F32 = mybir.dt.float32
BF = mybir.dt.bfloat16
NPBF = ml_dtypes.bfloat16

P = 128
NSEQ = 2048          # sequence length
CDIM = 2048          # model dim
HD = 64              # head dim
NHC = 8              # heads per core
KT = CDIM // P       # 16 contraction tiles
IB = 512             # i-block in attention
NIB = NSEQ // IB     # 4
JT = NSEQ // P       # 16 j-tiles
MF = 8               # qk f-tiles (0-3 q, 4-7 k)
NXC = 8              # x chunk tiles (2 kc each)
EXP_FUNC = mybir.ActivationFunctionType.Exp
SCALE = 1.0 / 8.0    # 1/sqrt(HD)

_CACHED_NC = None


def _build_nc():
    nc = bacc.Bacc(None)

    xt = nc.declare_dram_parameter("xt", [KT, P, NSEQ], BF, isOutput=False)
    wqkt = nc.declare_dram_parameter("wqkt", [P, KT, MF, P], BF, isOutput=False)
    wvt = nc.declare_dram_parameter("wvt", [P, KT, 512], BF, isOutput=False)
    wot = nc.declare_dram_parameter("wot", [P, 4, CDIM], BF, isOutput=False)
    cos2t = nc.declare_dram_parameter("cos2t", [P, NSEQ], F32, isOutput=False)
    sin2t = nc.declare_dram_parameter("sin2t", [P, NSEQ], F32, isOutput=False)
    r2t = nc.declare_dram_parameter("r2t", [P, P], BF, isOutput=False)
    onesv = nc.declare_dram_parameter("onesv", [P, JT, NHC, 1], BF, isOutput=False)
    onesb = nc.declare_dram_parameter("onesb", [P, HD], BF, isOutput=False)
    out_part = nc.declare_dram_parameter("out_part", [NSEQ, CDIM], F32, isOutput=True)

    n_repeat = int(os.environ.get("ATT_REPEAT", "1"))

    with tile.TileContext(nc) as tc, nc.allow_low_precision("bf16 matmul kernel"):
        for _rep in range(n_repeat):
            _kernel_body(nc, tc, xt, wqkt, wvt, wot, cos2t, sin2t, r2t,
                         onesv, onesb, out_part)

    nc.compile()
    return nc


def _kernel_body(nc, tc, xt, wqkt, wvt, wot, cos2t, sin2t, r2t,
                 onesv, onesb, out_part):
    with tc.tile_pool(name="persist", bufs=1) as persist:
        qk_sb = persist.tile([P, MF, NSEQ], BF)
        v1_sb = persist.tile([P, JT, NHC, HD + 1], BF)
        ones_sb = persist.tile([P, HD], BF)
        nc.sync.dma_start(out=ones_sb, in_=onesb[:, :])
        nc.sync.dma_start(out=v1_sb[:, :, :, HD:HD + 1], in_=onesv[:, :, :, :])

        # ---------------- Phase AB: projections + RoPE ----------------
        xp_cm = tc.tile_pool(name="xp", bufs=NXC)
        xp = xp_cm.__enter__()
        wp_cm = tc.tile_pool(name="wp", bufs=1)
        wp = wp_cm.__enter__()
        csp_cm = tc.tile_pool(name="csp", bufs=1)
        csp = csp_cm.__enter__()

        wvt_sb = wp.tile([P, KT, 512], BF)
        nc.sync.dma_start(out=wvt_sb, in_=wvt[:, :, :])
        xch = []
        for c in range(NXC):
            xc = xp.tile([P, 2, NSEQ], BF, tag="xc")
            for k in range(2):
                nc.sync.dma_start(out=xc[:, k, :], in_=xt[2 * c + k])
            xch.append(xc)
        wqk_sb = wp.tile([P, KT, MF, P], BF)
        nc.sync.dma_start(out=wqk_sb, in_=wqkt[:, :, :, :])
        cos_sb = csp.tile([P, NSEQ], F32)
        sin_sb = csp.tile([P, NSEQ], F32)
        r2_sb = csp.tile([P, P], BF)
        nc.sync.dma_start(out=cos_sb, in_=cos2t[:, :])
        nc.sync.dma_start(out=sin_sb, in_=sin2t[:, :])
        nc.sync.dma_start(out=r2_sb, in_=r2t[:, :])

        def xsl(kc, nslice):
            return xch[kc // 2][:, kc % 2, nslice]

        # v projection: [n on partitions, v-features free]
        with tc.tile_pool(name="psv", bufs=2, space="PSUM") as psv:
            for nt in range(JT):
                vp = psv.tile([P, 512], F32, tag="vp")
                for kc in range(KT):
                    nc.tensor.matmul(
                        vp,
                        xsl(kc, ts(nt, P)),
                        wvt_sb[:, kc, :],
                        start=(kc == 0),
                        stop=(kc == KT - 1),
                    )
                nc.scalar.copy(
                    out=v1_sb[:, nt, :, 0:HD],
                    in_=vp.rearrange("p (h d) -> p h d", h=NHC),
                )

        # q/k projection + RoPE: [qk-features on partitions, n free]
        with tc.tile_pool(name="psqk", bufs=2, space="PSUM") as psqk, \
             tc.tile_pool(name="psrot", bufs=2, space="PSUM") as psrot, \
             tc.tile_pool(name="stg", bufs=2) as stg, \
             tc.tile_pool(name="stg2", bufs=2) as stg2:
            for mf in range(MF):
                for nh in range(2):
                    nsl = ts(nh, 1024)
                    qp = psqk.tile([P, 2, 512], F32, tag="qp")
                    for kc in range(KT):
                        for q2 in range(2):
                            nc.tensor.matmul(
                                qp[:, q2, :],
                                wqk_sb[:, kc, mf, :],
                                xsl(kc, ts(2 * nh + q2, 512)),
                                start=(kc == 0),
                                stop=(kc == KT - 1),
                            )
                    qpf = qp.rearrange("p a b -> p (a b)")
                    qa = stg.tile([P, 1024], BF, tag="qa")
                    nc.scalar.copy(out=qa, in_=qpf)
                    rp = psrot.tile([P, 2, 512], F32, tag="rp")
                    for q2 in range(2):
                        nc.tensor.matmul(
                            rp[:, q2, :], r2_sb, qa[:, ts(q2, 512)],
                            start=True, stop=True,
                        )
                    t1 = stg2.tile([P, 1024], F32, tag="t1")
                    nc.vector.tensor_mul(out=t1, in0=qpf, in1=cos_sb[:, nsl])
                    t2 = stg2.tile([P, 1024], F32, tag="t2")
                    nc.vector.tensor_mul(
                        out=t2, in0=rp.rearrange("p a b -> p (a b)"),
                        in1=sin_sb[:, nsl],
                    )
                    nc.vector.tensor_add(out=qk_sb[:, mf, nsl], in0=t1, in1=t2)

        csp_cm.__exit__(None, None, None)
        wp_cm.__exit__(None, None, None)
        xp_cm.__exit__(None, None, None)

        # ---------------- Phase C: attention + output projection ----------------
        wotp_cm = tc.tile_pool(name="wotp", bufs=1)
        wotp = wotp_cm.__enter__()
        attp_cm = tc.tile_pool(name="attp", bufs=1)
        attp = attp_cm.__enter__()
        wot_sb = wotp.tile([P, 4, CDIM], BF)
        for ct in range(4):
            nc.sync.dma_start(out=wot_sb[:, ct, :], in_=wot[:, ct, :])
        att_sb = attp.tile([P, 4, NSEQ], BF)

        with tc.tile_pool(name="expp", bufs=10) as expp, \
             tc.tile_pool(name="smal", bufs=6) as smal, \
             tc.tile_pool(name="osb", bufs=3) as osb, \
             tc.tile_pool(name="pssc0", bufs=1, space="PSUM") as pssc0, \
             tc.tile_pool(name="pssc1", bufs=1, space="PSUM") as pssc1, \
             tc.tile_pool(name="psav", bufs=2, space="PSUM") as psav, \
             tc.tile_pool(name="psd", bufs=2, space="PSUM") as psd:

            def emit_outproj_quarter(ib, q):
                it = 4 * ib + q
                for ob in range(4):
                    op = psd.tile([P, 512], F32, tag="op")
                    for ct in range(4):
                        nc.tensor.matmul(
                            op,
                            att_sb[:, ct, ts(it, P)],
                            wot_sb[:, ct, ts(ob, 512)],
                            start=(ct == 0),
                            stop=(ct == 3),
                        )
                    o_sb = osb.tile([P, 512], F32, tag="osb")
                    nc.vector.tensor_copy(out=o_sb, in_=op)
                    nc.sync.dma_start(
                        out=out_part[ts(it, P), ts(ob, 512)], in_=o_sb
                    )

            def emit_scores_grp(t, isl, grp, exps):
                # two 2-bank psum tiles (head A row tile T0, head B row tile
                # T8); separate pools so exp(g) ping-pongs with scores(g+1)
                sc0 = pssc0.tile([P, 2, IB], F32, tag="sc0")
                sc1 = pssc1.tile([P, 2, IB], F32, tag="sc1")
                for j2 in range(2):
                    jt = 2 * grp + j2
                    nc.tensor.matmul(
                        sc0[:, j2, :],
                        qk_sb[0:HD, 4 + t, ts(jt, P)],
                        qk_sb[0:HD, t, isl],
                        start=True,
                        stop=True,
                    )
                    nc.tensor.matmul(
                        sc1[:, j2, :],
                        qk_sb[HD:P, 4 + t, ts(jt, P)],
                        qk_sb[HD:P, t, isl],
                        start=True,
                        stop=True,
                    )
                e0 = expp.tile([P, 2, IB], BF, tag="exp")
                nc.scalar.activation(out=e0, in_=sc0, func=EXP_FUNC, scale=SCALE)
                e1 = expp.tile([P, 2, IB], BF, tag="exp")
                nc.scalar.activation(out=e1, in_=sc1, func=EXP_FUNC, scale=SCALE)
                exps[grp] = (e0, e1)

            def emit_av_grp(t, grp, exps, avt):
                for h2 in range(2):
                    h = 2 * t + h2
                    for j2 in range(2):
                        jt = 2 * grp + j2
                        nc.tensor.matmul(
                            avt[h2],
                            v1_sb[:, jt, h, :],
                            exps[grp][h2][:, j2, :],
                            start=(jt == 0),
                            stop=(jt == JT - 1),
                            skip_group_check=True,
                        )

            def emit_normalize(t, isl, avt):
                # softmax normalize: denominator row 64 of av, broadcast by
                # ones outer product, approx reciprocal, scale
                for h2 in range(2):
                    hb = HD * h2
                    denb = smal.tile([HD + 1, IB], BF, tag="denb")
                    nc.vector.tensor_copy(
                        out=denb[HD:HD + 1, :],
                        in_=avt[h2][HD:HD + 1, :],
                    )
                    bc = psd.tile([P, IB], F32, tag="op")
                    nc.tensor.matmul(
                        bc[0:HD, :],
                        ones_sb[HD:HD + 1, :],
                        denb[HD:HD + 1, :],
                        start=True,
                        stop=True,
                    )
                    bcr = smal.tile([HD, IB], F32, tag="bcr")
                    nc.vector.reciprocal_approx_fast(out=bcr, in_=bc[0:HD, :])
                    nc.vector.tensor_mul(
                        out=att_sb[hb:hb + HD, t, isl],
                        in0=avt[h2][0:HD, :],
                        in1=bcr,
                    )

            # PE warm-up at the phase boundary: the AB tail leaves the PE
            # idle just long enough for the HAM to re-throttle the clock;
            # a few dependency-free streams keep the activity window busy.
            for _w in range(8):
                wu = psd.tile([P, IB], F32, tag="op")
                nc.tensor.matmul(
                    wu[0:HD, :],
                    ones_sb[HD:HD + 1, :],
                    qk_sb[HD:HD + 1, 0, 0:IB],
                    start=True,
                    stop=True,
                )

            prev = None  # (t, isl, exps, avt) pending av-tail + normalize
            for ib in range(NIB):
                isl = ts(ib, IB)
                for t in range(4):
                    exps = [None] * 8
                    emit_scores_grp(t, isl, 0, exps)
                    if prev is not None:
                        pt, pisl, pexps, pavt = prev
                        emit_av_grp(pt, 7, pexps, pavt)
                        emit_normalize(pt, pisl, pavt)
                    avt0 = psav.tile([HD + 1, IB], F32, tag="av")
                    avt1 = psav.tile([HD + 1, IB], F32, tag="av")
                    avt = (avt0, avt1)
                    for grp in range(1, 8):
                        emit_scores_grp(t, isl, grp, exps)
                        emit_av_grp(t, grp - 1, exps, avt)
                        # output projection of the previous i-block, spread
                        # one i-tile per head-pair slot
                        if grp == 3 and ib > 0:
                            emit_outproj_quarter(ib - 1, t)
                    prev = (t, isl, exps, avt)
            pt, pisl, pexps, pavt = prev
            emit_av_grp(pt, 7, pexps, pavt)
            emit_normalize(pt, pisl, pavt)
            for q in range(4):
                emit_outproj_quarter(NIB - 1, q)

        attp_cm.__exit__(None, None, None)
        wotp_cm.__exit__(None, None, None)


def _rot_matrix():
    r = np.zeros((HD, HD), dtype=np.float32)
    for d in range(32):
        r[d, d + 32] = -1.0
    for d in range(32, HD):
        r[d, d - 32] = 1.0
    r2 = np.zeros((P, P), dtype=np.float32)
    r2[0:HD, 0:HD] = r
    r2[HD:P, HD:P] = r
    return np.ascontiguousarray(r2.T).astype(NPBF)


def _core_inputs(x, cos_t, sin_t, W_qkv, W_out, core):
    b, g = divmod(core, 4)
    hs = g * NHC * HD  # feature offset of this head group (512 per group)

    xT = np.ascontiguousarray(x[b].T).astype(NPBF)  # [c, n]
    xt = np.ascontiguousarray(xT.reshape(KT, P, NSEQ))

    Wq = W_qkv[hs:hs + 512]
    Wk = W_qkv[CDIM + hs:CDIM + hs + 512]
    Wv = W_qkv[2 * CDIM + hs:2 * CDIM + hs + 512]
    WqkT = np.concatenate([Wq, Wk], axis=0).T  # [c, 1024]
    wqkt = np.ascontiguousarray(
        WqkT.reshape(KT, P, MF, P).transpose(1, 0, 2, 3)
    ).astype(NPBF)
    WvT = Wv.T  # [c, 512]
    wvt = np.ascontiguousarray(
        WvT.reshape(KT, P, 512).transpose(1, 0, 2)
    ).astype(NPBF)
    WoT = W_out[:, hs:hs + 512].T  # [c-slice 512, o 2048]
    wot = np.ascontiguousarray(
        WoT.reshape(4, P, CDIM).transpose(1, 0, 2)
    ).astype(NPBF)

    return {
        "xt": xt,
        "wqkt": wqkt,
        "wvt": wvt,
        "wot": wot,
        "cos2t": cos_t,
        "sin2t": sin_t,
        "r2t": _ROT,
        "onesv": _ONESV,
        "onesb": _ONESB,
    }


_ROT = _rot_matrix()
_ONESV = np.ones((P, JT, NHC, 1), dtype=NPBF)
_ONESB = np.ones((P, HD), dtype=NPBF)


def kernel(x, freqs, W_qkv, W_out):
    global _CACHED_NC
    x = np.asarray(x, dtype=np.float32)
    freqs = np.asarray(freqs, dtype=np.float32)
    W_qkv = np.asarray(W_qkv, dtype=np.float32)
    W_out = np.asarray(W_out, dtype=np.float32)

    if _CACHED_NC is None:
        _CACHED_NC = _build_nc()
    nc = _CACHED_NC

    cos_t = np.ascontiguousarray(np.tile(np.cos(freqs.T), (2, 1)))  # [128, n]
    sin_t = np.ascontiguousarray(np.tile(np.sin(freqs.T), (2, 1)))

    in_maps = [
        _core_inputs(x, cos_t, sin_t, W_qkv, W_out, core) for core in range(8)
    ]
    trace = os.environ.get("ATT_TRACE") == "1"
    tdir = os.environ.get("ATT_TRACE_DIR") or None
    res = run_bass_kernel_spmd(
        nc, in_maps, core_ids=list(range(8)), trace=trace, tmpdir=tdir
    )
    if trace and res.exec_time_ns is not None:
        print(f"HW exec time: {res.exec_time_ns} ns")

    out = np.empty((2, NSEQ, CDIM), dtype=np.float32)
    for b in range(2):
        acc = np.zeros((NSEQ, CDIM), dtype=np.float64)
        for g in range(4):
            acc += res.results[4 * b + g]["out_part"]
        out[b] = acc.astype(np.float32)
    return out
